# revision 1
# baseline (speedup 1.0000x reference)
"""Causal self-attention (GQA + QK-RMSNorm + RoPE + q_gain) on 8 Trainium2 cores.

Sharding: 8 cores = 2 (batch) x 4 (KV head group).  Core c handles batch
c//4 and KV head g=c%4, i.e. Q heads 4g..4g+3.  Each core computes its
heads' attention and a partial output projection (its 512 columns of the
attention output against the matching 512 rows of Wproj^T); the host sums
the 4 partials per batch.

All shapes are hardcoded for B=2, S=2048, D=2048, H=16, KVH=4, HD=128.
Matmuls run as float32r (full PE rate at N>=256, ~tf32 precision), fp32
storage everywhere.
"""

import numpy as np

B, S, D = 2, 2048, 2048
H, KVH = 16, 4
HD = 128  # head dim
G = H // KVH  # q heads per kv group = 4
NCORES = 8
ROPE_BASE = 10000.0
EPS = 1e-6

P = 128          # partitions
SL = 512         # token slice for phase 1 / proj
NSL = S // SL    # 4
DK = D // P      # 16 contraction subtiles
NMEGA = 2        # attention processes sq in mega-slices of 1024
MEGA = 1024

_CACHE = {}


def _build_program():
    """Build + compile the (single, SPMD) Bass program. Returns nc."""
    from contextlib import ExitStack

    import concourse.bass as bass
    import concourse.tile as tile
    from concourse import bacc, mybir
    from concourse.masks import make_identity

    f32 = mybir.dt.float32
    f32r = mybir.dt.float32r
    AF = mybir.ActivationFunctionType
    OP = mybir.AluOpType

    nc = bacc.Bacc("TRN2", target_bir_lowering=False)

    xT_d = nc.dram_tensor("xT", [D, S], f32r, kind="ExternalInput").ap()
    wqT_d = nc.dram_tensor("wqT", [D, G * HD], f32r, kind="ExternalInput").ap()
    wkT_d = nc.dram_tensor("wkT", [D, HD], f32r, kind="ExternalInput").ap()
    wvT_d = nc.dram_tensor("wvT", [D, HD], f32r, kind="ExternalInput").ap()
    wpT_d = nc.dram_tensor("wpT", [G * HD, D], f32r, kind="ExternalInput").ap()
    cosT_d = nc.dram_tensor("cosT", [HD, S], f32, kind="ExternalInput").ap()
    sinT_d = nc.dram_tensor("sinT", [HD, S], f32, kind="ExternalInput").ap()
    jT_d = nc.dram_tensor("jT", [HD, HD], f32r, kind="ExternalInput").ap()
    qgain_d = nc.dram_tensor("qgain", [1, G], f32, kind="ExternalInput").ap()
    y_d = nc.dram_tensor("y", [S, D], f32, kind="ExternalOutput").ap()

    with tile.TileContext(nc) as tc, ExitStack() as top:
        res = top.enter_context(tc.tile_pool(name="resident", bufs=1))

        # ---- small constants ----
        ones_f = res.tile([P, P], f32)
        nc.vector.memset(ones_f[:], 1.0)
        ones_mat = res.tile([P, P], f32r)
        nc.vector.tensor_copy(ones_mat[:], ones_f[:])
        ident = res.tile([P, P], f32)
        make_identity(nc, ident[:])
        eps_t = res.tile([P, 1], f32)
        nc.vector.memset(eps_t[:], EPS)
        qgain = res.tile([P, G], f32)
        nc.gpsimd.dma_start(qgain[:], qgain_d.to_broadcast([P, G]))
        jT = res.tile([HD, HD], f32r)
        nc.sync.dma_start(jT[:], jT_d[:])

        # ---- resident Q^T/K^T/V ----
        qT = [res.tile([P, S], f32r, tag=f"qT{h}", name=f"qT{h}") for h in range(G)]
        kT = res.tile([P, S], f32r)
        v_sb = res.tile([P, S // P, HD], f32r)  # V natural, [s_inner, s_tile, hd]

        # ================= PHASE 1: QKV + RMSNorm + RoPE =================
        with ExitStack() as ph1:
            wq = ph1.enter_context(tc.tile_pool(name="wq", bufs=1))
            xtp = ph1.enter_context(tc.tile_pool(name="xt", bufs=24))
            tmp = ph1.enter_context(tc.tile_pool(name="p1tmp", bufs=3))
            rowp = ph1.enter_context(tc.tile_pool(name="p1row", bufs=3))
            csp = ph1.enter_context(tc.tile_pool(name="cs", bufs=1))
            ps1 = ph1.enter_context(tc.tile_pool(name="ps1", bufs=5, space="PSUM"))
            psj = ph1.enter_context(tc.tile_pool(name="psj", bufs=1, space="PSUM"))
            pssq = ph1.enter_context(tc.tile_pool(name="pssq", bufs=1, space="PSUM"))
            pstr = ph1.enter_context(tc.tile_pool(name="pstr", bufs=1, space="PSUM"))

            xT3 = xT_d.rearrange("(o p) s -> p o s", p=P)
            HK = DK // 16  # xt loaded per-dk

            # DMA issue order matters at startup: K/V weights and the first
            # x-slice first (PE starts on K), the big Wq load last.
            wk_sb = wq.tile([P, DK, HD], f32r)
            nc.sync.dma_start(wk_sb[:], wkT_d.rearrange("(o p) m -> p o m", p=P))
            xth0 = [xtp.tile([P, HK, SL], f32r, tag="xt", name=f"xt_0_{c}")
                    for c in range(16)]
            nc.sync.dma_start(xth0[0][:], xT3[:, 0:HK, 0:SL])
            wv_sb = wq.tile([P, DK, HD], f32r)
            nc.sync.dma_start(wv_sb[:], wvT_d.rearrange("(o p) m -> p o m", p=P))
            for c in range(1, 16):
                nc.sync.dma_start(xth0[c][:], xT3[:, c * HK:(c + 1) * HK, 0:SL])
            wq_sb = wq.tile([P, DK, G * HD], f32r)
            wqT3 = wqT_d.rearrange("(o p) m -> p o m", p=P)
            for h in range(G):
                nc.sync.dma_start(wq_sb[:, :, h * HD:(h + 1) * HD],
                                  wqT3[:, :, h * HD:(h + 1) * HD])
            cos_sb = csp.tile([HD, S], f32)
            nc.sync.dma_start(cos_sb[:], cosT_d[:])
            sin_sb = csp.tile([HD, S], f32)
            nc.sync.dma_start(sin_sb[:], sinT_d[:])

            def norm_rope(src_ps, dst, js, gain_ap):
                """RMS-normalize (+optional gain) and RoPE a [128, SL] head block.

                src_ps: PSUM tile [P, SL] holding raw projection output.
                dst: SBUF AP [P, SL] (slice of resident q/k tile).
                gain_ap: [1,1] AP with gain/sqrt(HD) premultiplied, or None.
                """
                sq = tmp.tile([P, SL], f32r, tag="sq")
                nc.scalar.square(sq[:], src_ps[:])
                # ones_mat gives the column sums replicated on all 128
                # partitions (same matmul cost -- cost is N-bound), so the
                # per-token norm factor needs no partition broadcast
                ssq = pssq.tile([P, SL], f32, tag="ssq")
                nc.tensor.matmul(ssq[:], ones_mat[:], sq[:], start=True,
                                 stop=True)
                # f = gain/sqrt(ssq/HD + eps)
                fb = tmp.tile([P, SL], f32, tag="fb")
                nc.scalar.activation(fb[:], ssq[:], AF.Sqrt,
                                     bias=eps_t[:], scale=1.0 / HD)
                nc.vector.reciprocal(fb[:], fb[:])
                if gain_ap is not None:
                    nc.vector.tensor_scalar_mul(fb[:], fb[:], gain_ap)
                qn = tmp.tile([P, SL], f32r, tag="qn")
                nc.vector.tensor_mul(qn[:], src_ps[:], fb[:])
                # rope: rot(q) = qn*cosf + (J @ qn)*sinf, J = [[0,I],[-I,0]]
                qj = psj.tile([P, SL], f32, tag="qj")
                nc.tensor.matmul(qj[:], jT[:], qn[:], start=True, stop=True)
                c = cos_sb[:, js * SL:(js + 1) * SL]
                s = sin_sb[:, js * SL:(js + 1) * SL]
                t1 = tmp.tile([P, SL], f32, tag="t1")
                t2 = tmp.tile([P, SL], f32, tag="t2")
                nc.vector.tensor_mul(t1[:], qn[:], c)
                nc.vector.tensor_mul(t2[:], qj[:], s)
                nc.vector.tensor_add(dst, t1[:], t2[:])

            for js in range(NSL):
                if js == 0:
                    xth = xth0
                else:
                    xth = [xtp.tile([P, HK, SL], f32r, tag="xt",
                                    name=f"xt_{js}_{c}") for c in range(16)]
                    for c in range(16):
                        nc.sync.dma_start(
                            xth[c][:],
                            xT3[:, c * HK:(c + 1) * HK, js * SL:(js + 1) * SL])

                def xts(dk):
                    return xth[dk // HK][:, dk % HK, :]

                # K first (small weights -> PE starts early).  For js=0 the
                # x chunks are still streaming in, and the PE executes its
                # queue in order -- so interleave the K/V accumulation groups
                # over dk so each arriving chunk feeds two matmuls at once
                # instead of K's tail chunk blocking all of V.
                k_ps = ps1.tile([P, SL], f32, tag="qkv")
                v_ps = ps1.tile([P, SL], f32, tag="qkv")
                if js == 0:
                    q0_ps = ps1.tile([P, SL], f32, tag="qkv", name="q0i_ps")
                    q1_ps = ps1.tile([P, SL], f32, tag="qkv", name="q1i_ps")
                    for dk in range(DK):
                        nc.tensor.matmul(k_ps[:], wk_sb[:, dk, :], xts(dk),
                                         start=(dk == 0), stop=(dk == DK - 1))
                        nc.tensor.matmul(v_ps[:], wv_sb[:, dk, :], xts(dk),
                                         start=(dk == 0), stop=(dk == DK - 1))
                        nc.tensor.matmul(q0_ps[:], wq_sb[:, dk, 0:HD], xts(dk),
                                         start=(dk == 0), stop=(dk == DK - 1))
                        nc.tensor.matmul(q1_ps[:], wq_sb[:, dk, HD:2 * HD],
                                         xts(dk),
                                         start=(dk == 0), stop=(dk == DK - 1))
                else:
                    for dk in range(DK):
                        nc.tensor.matmul(k_ps[:], wk_sb[:, dk, :], xts(dk),
                                         start=(dk == 0), stop=(dk == DK - 1))
                    for dk in range(DK):
                        nc.tensor.matmul(v_ps[:], wv_sb[:, dk, :], xts(dk),
                                         start=(dk == 0), stop=(dk == DK - 1))
                norm_rope(k_ps, kT[:, js * SL:(js + 1) * SL], js, None)
                vt = tmp.tile([P, SL], f32, tag="vt")
                nc.vector.tensor_copy(vt[:], v_ps[:])
                for t in range(SL // P):
                    vtr_ps = pstr.tile([P, P], f32, tag="vtr")
                    nc.tensor.transpose(vtr_ps[:], vt[:, t * P:(t + 1) * P], ident[:])
                    nc.scalar.copy(v_sb[:, js * (SL // P) + t, :], vtr_ps[:])
                # Q heads
                for h in range(G):
                    if js == 0 and h == 0:
                        q_ps = q0_ps
                    elif js == 0 and h == 1:
                        q_ps = q1_ps
                    else:
                        q_ps = ps1.tile([P, SL], f32, tag="qkv")
                        for dk in range(DK):
                            nc.tensor.matmul(
                                q_ps[:], wq_sb[:, dk, h * HD:(h + 1) * HD],
                                xts(dk), start=(dk == 0), stop=(dk == DK - 1))
                    norm_rope(q_ps, qT[h][:, js * SL:(js + 1) * SL], js,
                              qgain[:, h:h + 1])

        # ================= PHASE 2: attention + proj =====================
        with ExitStack() as ph2:
            wpp = ph2.enter_context(tc.tile_pool(name="wp", bufs=1))
            otp = ph2.enter_context(tc.tile_pool(name="oT", bufs=1))
            ptp = ph2.enter_context(tc.tile_pool(name="pt", bufs=6))
            rowp2 = ph2.enter_context(tc.tile_pool(name="p2row", bufs=3))
            bb = ph2.enter_context(tc.tile_pool(name="p2b", bufs=2))

            wp_sb = wpp.tile([P, G, D], f32r)
            nc.sync.dma_start(wp_sb[:], wpT_d.rearrange("(o p) m -> p o m", p=P))
            oT = [otp.tile([P, S], f32r, tag=f"oT{h}", name=f"oT{h}") for h in range(G)]

            with ExitStack() as attn:
                pssc = attn.enter_context(
                    tc.tile_pool(name="pssc", bufs=5, space="PSUM"))
                pso = attn.enter_context(
                    tc.tile_pool(name="pso", bufs=2, space="PSUM"))
                psrs = attn.enter_context(
                    tc.tile_pool(name="psrs", bufs=1, space="PSUM"))

                for h in range(G):
                    for jq in range(NSL):
                        o_ps = pso.tile([P, SL], f32, tag="o")
                        rs_ps = psrs.tile([P, SL], f32, tag="rs")
                        ilast = 4 * jq + 3       # last key tile index
                        for i in range(ilast + 1):
                            delta = i - 4 * jq
                            # diagonal blocks: only columns >= 128*delta can
                            # be causally valid; restrict everything (scores,
                            # exp, mask, rowsum, PV) to that span.  i==0 is
                            # always full width, so the PSUM accumulation
                            # groups start full.
                            lo_c = P * delta if 0 <= delta <= 3 else 0
                            sp = slice(lo_c, SL)
                            qsp = slice(jq * SL + lo_c, (jq + 1) * SL)
                            sc = pssc.tile([P, SL], f32, tag="sc")
                            nc.tensor.matmul(sc[:, sp],
                                             kT[:, i * P:(i + 1) * P],
                                             qT[h][:, qsp],
                                             start=True, stop=True)
                            pt = ptp.tile([P, SL], f32r, tag="pt")
                            nc.scalar.activation(pt[:, sp], sc[:, sp], AF.Exp)
                            if 0 <= delta <= 3:
                                nc.gpsimd.affine_select(
                                    out=pt[:, sp], in_=pt[:, sp],
                                    compare_op=OP.is_ge, fill=0.0,
                                    base=0, pattern=[[1, SL - lo_c]],
                                    channel_multiplier=-1)
                            stop = i == ilast
                            nc.tensor.matmul(rs_ps[:, sp], ones_mat[:],
                                             pt[:, sp],
                                             start=(i == 0), stop=stop)
                            nc.tensor.matmul(o_ps[:, sp], v_sb[:, i, :],
                                             pt[:, sp],
                                             start=(i == 0), stop=stop)
                        # normalize: oT = o_ps / rowsum (rs_ps rows are
                        # already the replicated row-sums)
                        rb = bb.tile([P, SL], f32, tag="rb")
                        nc.vector.reciprocal(rb[:], rs_ps[:])
                        nc.vector.tensor_mul(
                            oT[h][:, jq * SL:(jq + 1) * SL], o_ps[:], rb[:])

            # ---- output projection: y[s,o] partial ----
            with ExitStack() as proj:
                psy = proj.enter_context(
                    tc.tile_pool(name="psy", bufs=4, space="PSUM"))
                yout = proj.enter_context(tc.tile_pool(name="yout", bufs=2))
                for st in range(S // P):
                    y_sb = yout.tile([P, D], f32, tag="ysb")
                    for os_ in range(D // SL):
                        y_ps = psy.tile([P, SL], f32, tag="y")
                        for h in range(G):
                            nc.tensor.matmul(
                                y_ps[:], oT[h][:, st * P:(st + 1) * P],
                                wp_sb[:, h, os_ * SL:(os_ + 1) * SL],
                                start=(h == 0), stop=(h == G - 1))
                        if os_ % 2 == 0:
                            nc.scalar.copy(y_sb[:, os_ * SL:(os_ + 1) * SL],
                                           y_ps[:])
                        else:
                            nc.vector.tensor_copy(
                                y_sb[:, os_ * SL:(os_ + 1) * SL], y_ps[:])
                    for yh in range(4):
                        nc.sync.dma_start(
                            y_d[st * P:(st + 1) * P,
                                yh * (D // 4):(yh + 1) * (D // 4)],
                            y_sb[:, yh * (D // 4):(yh + 1) * (D // 4)])

    nc.compile()
    return nc


def _rope_tables():
    """cos/sin tables in [HD, S] layout (half-tables stacked twice), plus J^T."""
    inv_freq = 1.0 / (ROPE_BASE ** (np.arange(0, HD, 2, dtype=np.float32) / HD))
    freqs = np.outer(np.arange(S, dtype=np.float32), inv_freq)  # [S, half]
    c = np.cos(freqs).T.astype(np.float32)  # [half, S]
    s = np.sin(freqs).T.astype(np.float32)
    cosf = np.concatenate([c, c], axis=0).copy()  # [HD, S]
    sinf = np.concatenate([s, s], axis=0).copy()
    half = HD // 2
    jT = np.zeros((HD, HD), np.float32)
    jT[np.arange(half) + half, np.arange(half)] = 1.0   # (Jq)[j] = q[j+64], j<64
    jT[np.arange(half), np.arange(half) + half] = -1.0  # (Jq)[j+64] = -q[j]
    return cosf, sinf, jT


def make_in_maps(x, Wq, Wk, Wv, Wproj, q_gain):
    """Host-side shard prep: per-core input dicts."""
    cosT, sinT, jT = _rope_tables()
    xT = np.ascontiguousarray(np.transpose(np.asarray(x, np.float32), (0, 2, 1)))
    Wq = np.asarray(Wq, np.float32)
    Wk = np.asarray(Wk, np.float32)
    Wv = np.asarray(Wv, np.float32)
    WpT = np.ascontiguousarray(np.asarray(Wproj, np.float32).T)  # [in, out]
    q_gain = np.asarray(q_gain, np.float32)

    in_maps = []
    for c in range(NCORES):
        b, g = divmod(c, KVH)
        sl_q = slice(g * G * HD, (g + 1) * G * HD)
        sl_kv = slice(g * HD, (g + 1) * HD)
        in_maps.append({
            "xT": xT[b],
            "wqT": np.ascontiguousarray(Wq[sl_q, :].T),
            "wkT": np.ascontiguousarray(Wk[sl_kv, :].T),
            "wvT": np.ascontiguousarray(Wv[sl_kv, :].T),
            "wpT": np.ascontiguousarray(WpT[sl_q, :]),
            "cosT": cosT,
            "sinT": sinT,
            "jT": jT,
            "qgain": (q_gain[g * G:(g + 1) * G] / np.sqrt(HD))
            .reshape(1, G).astype(np.float32),
        })
    return in_maps


def kernel(x, Wq, Wk, Wv, Wproj, q_gain):
    from concourse.bass_utils import run_bass_kernel_spmd

    if "nc" not in _CACHE:
        _CACHE["nc"] = _build_program()
    nc = _CACHE["nc"]

    in_maps = make_in_maps(x, Wq, Wk, Wv, Wproj, q_gain)
    res = run_bass_kernel_spmd(nc, in_maps, core_ids=list(range(NCORES)))
    _CACHE["last_results"] = res

    y = np.zeros((B, S, D), dtype=np.float32)
    for c in range(NCORES):
        y[c // KVH] += res.results[c]["y"]
    return y



# revision 6
# speedup vs baseline: 1.0152x; 1.0152x over previous
"""Causal self-attention (GQA + QK-RMSNorm + RoPE + q_gain) on 8 Trainium2 cores.

Sharding: 8 cores = 2 (batch) x 4 (KV head group).  Core c handles batch
c//4 and KV head g=c%4, i.e. Q heads 4g..4g+3.  Each core computes its
heads' attention and a partial output projection (its 512 columns of the
attention output against the matching 512 rows of Wproj^T); the host sums
the 4 partials per batch.

Implementation notes (v2, fp8/bf16 mixed precision):
- QKV projections run as 3-term fp8e4m3 DoubleRow matmuls: x ~ x8 + xr8
  (quantized host-side at scale 2^4), W ~ w8 + wr8 (scale 2^9), and the
  x8@w8 + x8@wr8 + xr8@w8 terms accumulate in PSUM (xr8@wr8 ~ 0.06% is
  dropped).  DoubleRow packs two 128-deep contraction tiles per
  instruction at half cost.  The 2^13 product scale cancels inside
  QK-RMSNorm; for V it is folded into Wproj on the host.
- V is produced directly in [token, hd] layout (DoubleRow, N=128), so no
  PE transposes are needed.
- Attention scores run in f32r; exp writes bf16 pt tiles; the softmax
  denominator accumulates pt tiles on the DVE (bf16, 2x rate) and needs
  only ONE ones-matmul per (head, query-slice) instead of one per key
  tile.  PV and the output projection run in bf16 (full PE rate).
- Causal diagonal blocks are padded to a moving size >= 256 (f32r matmuls
  below 256 run at 1/4 rate).
- Single merged pipeline: QKV(t) -> attention(t), with projection(t-1)
  matmuls interleaved into the attention stream as PE filler while the
  activation engine streams exps.

All shapes hardcoded for B=2, S=2048, D=2048, H=16, KVH=4, HD=128.
"""

import numpy as np

B, S, D = 2, 2048, 2048
H, KVH = 16, 4
HD = 128  # head dim
G = H // KVH  # q heads per kv group = 4
NCORES = 8
ROPE_BASE = 10000.0
EPS = 1e-6

P = 128          # partitions
SL = 512         # token slice
NSL = S // SL    # 4
DK = D // P      # 16 contraction subtiles
NPAIR = DK // 2  # 8 DoubleRow pairs

SX = 2.0 ** 4    # x quantization scale
SW = 2.0 ** 9    # weight quantization scale
SXW = SX * SW    # product scale 2^13

_CACHE = {}


def _build_program():
    """Build + compile the (single, SPMD) Bass program. Returns nc."""
    from contextlib import ExitStack

    import concourse.bass as bass
    import concourse.tile as tile
    from concourse import bacc, mybir

    f32 = mybir.dt.float32
    f32r = mybir.dt.float32r
    f8e4 = mybir.dt.float8e4
    bf16 = mybir.dt.bfloat16
    AF = mybir.ActivationFunctionType
    OP = mybir.AluOpType
    DR = mybir.MatmulPerfMode.DoubleRow

    nc = bacc.Bacc("TRN2", target_bir_lowering=False)

    x8_d = nc.dram_tensor("x8", [D, S], f8e4, kind="ExternalInput").ap()
    xr8_d = nc.dram_tensor("xr8", [D, S], f8e4, kind="ExternalInput").ap()
    wq8_d = nc.dram_tensor("wq8", [D, G * HD], f8e4, kind="ExternalInput").ap()
    wqr8_d = nc.dram_tensor("wqr8", [D, G * HD], f8e4, kind="ExternalInput").ap()
    wk8_d = nc.dram_tensor("wk8", [D, HD], f8e4, kind="ExternalInput").ap()
    wkr8_d = nc.dram_tensor("wkr8", [D, HD], f8e4, kind="ExternalInput").ap()
    wv8_d = nc.dram_tensor("wv8", [D, HD], f8e4, kind="ExternalInput").ap()
    wvr8_d = nc.dram_tensor("wvr8", [D, HD], f8e4, kind="ExternalInput").ap()
    wpT_d = nc.dram_tensor("wpT", [G * HD, D], bf16, kind="ExternalInput").ap()
    cosT_d = nc.dram_tensor("cosT", [HD, S], f32, kind="ExternalInput").ap()
    sinT_d = nc.dram_tensor("sinT", [HD, S], f32, kind="ExternalInput").ap()
    jT_d = nc.dram_tensor("jT", [HD, HD], f32r, kind="ExternalInput").ap()
    qgain_d = nc.dram_tensor("qgain", [1, G], f32, kind="ExternalInput").ap()
    y_d = nc.dram_tensor("y", [S, D], f32, kind="ExternalOutput").ap()

    x8_3 = x8_d.rearrange("(o p) s -> p o s", p=P)
    xr8_3 = xr8_d.rearrange("(o p) s -> p o s", p=P)

    with tile.TileContext(nc) as tc, ExitStack() as top:
        res = top.enter_context(tc.tile_pool(name="resident", bufs=1))
        xtp = top.enter_context(tc.tile_pool(name="xt", bufs=2))
        tmp = top.enter_context(tc.tile_pool(name="tmp", bufs=3))
        ptp = top.enter_context(tc.tile_pool(name="pt", bufs=4))
        ptsp = top.enter_context(tc.tile_pool(name="ptsum", bufs=3))
        rbp = top.enter_context(tc.tile_pool(name="rb", bufs=2))
        ysp = top.enter_context(tc.tile_pool(name="ysb", bufs=4))

        ps_big = top.enter_context(tc.tile_pool(name="psbig", bufs=2, space="PSUM"))
        ps_sc = top.enter_context(tc.tile_pool(name="pssc", bufs=2, space="PSUM"))
        ps_o = top.enter_context(tc.tile_pool(name="pso", bufs=2, space="PSUM"))
        ps_nm = top.enter_context(tc.tile_pool(name="psnm", bufs=1, space="PSUM"))
        ps_vo = top.enter_context(tc.tile_pool(name="psvo", bufs=1, space="PSUM"))

        # ---- startup DMAs: small K/V weights + first x chunks first ----
        wk8 = res.tile([P, DK, HD], f8e4)
        nc.sync.dma_start(wk8[:], wk8_d.rearrange("(o p) m -> p o m", p=P))
        xts0 = xtp.tile([P, DK, SL], f8e4, tag="x8", name="x8_0")
        for c in range(DK):
            nc.sync.dma_start(xts0[:, c, :], x8_3[:, c, 0:SL])
        wv8 = res.tile([P, DK, HD], f8e4)
        nc.sync.dma_start(wv8[:], wv8_d.rearrange("(o p) m -> p o m", p=P))
        wq8 = res.tile([P, DK, G * HD], f8e4)
        nc.sync.dma_start(wq8[:], wq8_d.rearrange("(o p) m -> p o m", p=P))
        xrs0 = xtp.tile([P, DK, SL], f8e4, tag="xr8", name="xr8_0")
        for c in range(DK):
            nc.sync.dma_start(xrs0[:, c, :], xr8_3[:, c, 0:SL])
        wkr8 = res.tile([P, DK, HD], f8e4)
        nc.sync.dma_start(wkr8[:], wkr8_d.rearrange("(o p) m -> p o m", p=P))
        wvr8 = res.tile([P, DK, HD], f8e4)
        nc.sync.dma_start(wvr8[:], wvr8_d.rearrange("(o p) m -> p o m", p=P))
        wqr8 = res.tile([P, DK, G * HD], f8e4)
        nc.sync.dma_start(wqr8[:], wqr8_d.rearrange("(o p) m -> p o m", p=P))
        cos_sb = res.tile([HD, S], f32)
        nc.sync.dma_start(cos_sb[:], cosT_d[:])
        sin_sb = res.tile([HD, S], f32)
        nc.sync.dma_start(sin_sb[:], sinT_d[:])
        jT = res.tile([HD, HD], f32r)
        nc.sync.dma_start(jT[:], jT_d[:])
        wp_sb = res.tile([P, G, D], bf16)
        nc.sync.dma_start(wp_sb[:], wpT_d.rearrange("(o p) m -> p o m", p=P))
        qgain = res.tile([P, G], f32)
        nc.gpsimd.dma_start(qgain[:], qgain_d.to_broadcast([P, G]))

        # ---- small constants ----
        ones_f = res.tile([P, P], f32)
        nc.vector.memset(ones_f[:], 1.0)
        ones_r = res.tile([P, P], f32r)
        nc.vector.tensor_copy(ones_r[:], ones_f[:])
        ones_b = res.tile([P, P], bf16)
        nc.vector.tensor_copy(ones_b[:], ones_f[:])
        eps_t = res.tile([P, 1], f32)
        nc.vector.memset(eps_t[:], EPS * SXW * SXW)  # eps * 2^26

        # ---- resident tensors ----
        qT = [res.tile([P, S], f32r, tag=f"qT{h}", name=f"qT{h}") for h in range(G)]
        kT = res.tile([P, S], f32r)
        v_sb = res.tile([P, S // P, HD], bf16)  # natural [s_inner, s_tile, hd]
        oT = [res.tile([P, S], bf16, tag=f"oT{h}", name=f"oT{h}") for h in range(G)]

        def qkv_group(out_ps, w8t, wr8t, x8t, xr8t, mlo, mhi, n):
            """3-term fp8 DoubleRow accumulation: (x8+xr8)@(w8+wr8) minus
            the xr8@wr8 term.  24 matmuls into one PSUM group.
            Term order: x8@w8 first (its inputs arrive earliest)."""
            terms = [(w8t, x8t), (wr8t, x8t), (w8t, xr8t)]
            first, last = (0, 0), (len(terms) - 1, NPAIR - 1)
            for ti, (wt, xt) in enumerate(terms):
                for pr in range(NPAIR):
                    nc.tensor.matmul(
                        out_ps[:, 0:n],
                        wt[:, 2 * pr:2 * pr + 2, mlo:mhi],
                        xt[:, 2 * pr:2 * pr + 2, 0:n],
                        start=(ti, pr) == first, stop=(ti, pr) == last,
                        perf_mode=DR)

        def norm_rope(src_ps, dst, js, gain_ap):
            """RMS-normalize (+optional gain) and RoPE a [128, SL] head block.

            src_ps holds the raw fp8-path projection at scale 2^13; the
            norm factor is computed at the same scale so qn comes out at
            true scale (eps folded in at 2^26)."""
            sq = tmp.tile([P, SL], f32r, tag="sq")
            nc.scalar.square(sq[:], src_ps[:])
            # ones_r matmul -> column sums replicated on all partitions
            ssq = ps_nm.tile([P, SL], f32, tag="nm", name="ssq")
            nc.tensor.matmul(ssq[:], ones_r[:], sq[:], start=True, stop=True)
            # fb = sqrt(ssq/HD + eps*2^26) = 2^13 * rms_true
            fb = tmp.tile([P, SL], f32, tag="fb")
            nc.scalar.activation(fb[:], ssq[:], AF.Sqrt,
                                 bias=eps_t[:], scale=1.0 / HD)
            nc.vector.reciprocal(fb[:], fb[:])
            if gain_ap is not None:
                nc.vector.tensor_scalar_mul(fb[:], fb[:], gain_ap)
            qn = tmp.tile([P, SL], f32r, tag="qn")
            nc.vector.tensor_mul(qn[:], src_ps[:], fb[:])
            # rope: rot(q) = qn*cosf + (J @ qn)*sinf, J = [[0,I],[-I,0]]
            qj = ps_nm.tile([P, SL], f32, tag="nm", name="qj")
            nc.tensor.matmul(qj[:], jT[:], qn[:], start=True, stop=True)
            c = cos_sb[:, js * SL:(js + 1) * SL]
            s = sin_sb[:, js * SL:(js + 1) * SL]
            t1 = tmp.tile([P, SL], f32, tag="t1")
            t2 = tmp.tile([P, SL], f32, tag="t2")
            nc.gpsimd.tensor_mul(t1[:], qn[:], c)
            nc.vector.tensor_mul(t2[:], qj[:], s)
            nc.vector.tensor_add(dst, t1[:], t2[:])

        # projection work items, generated per finished jq slice and
        # interleaved into the next slice's attention stream as PE filler
        def proj_chunk(jq, st, os_):
            st_g = jq * (SL // P) + st
            y_ps = ps_big.tile([P, SL], f32, tag="big", name=f"y_{st_g}_{os_}")
            for h in range(G):
                nc.tensor.matmul(
                    y_ps[:], oT[h][:, st_g * P:(st_g + 1) * P],
                    wp_sb[:, h, os_ * SL:(os_ + 1) * SL],
                    start=(h == 0), stop=(h == G - 1))
            y_sb = ysp.tile([P, SL], f32, tag="ysb")
            if os_ % 2 == 0:
                nc.scalar.copy(y_sb[:], y_ps[:])
            else:
                nc.vector.tensor_copy(y_sb[:], y_ps[:])
            nc.sync.dma_start(
                y_d[st_g * P:(st_g + 1) * P, os_ * SL:(os_ + 1) * SL], y_sb[:])

        for js in range(NSL):
            # ---- x chunks for this slice (js=0 preloaded above) ----
            if js == 0:
                x8t, xr8t = xts0, xrs0
            else:
                x8t = xtp.tile([P, DK, SL], f8e4, tag="x8", name=f"x8_{js}")
                xr8t = xtp.tile([P, DK, SL], f8e4, tag="xr8", name=f"xr8_{js}")
                for c in range(DK):
                    nc.sync.dma_start(x8t[:, c, :],
                                      x8_3[:, c, js * SL:(js + 1) * SL])
                for c in range(DK):
                    nc.sync.dma_start(xr8t[:, c, :],
                                      xr8_3[:, c, js * SL:(js + 1) * SL])

            # ================= QKV for slice js =================
            k_ps = ps_big.tile([P, SL], f32, tag="big", name=f"k_ps{js}")
            qkv_group(k_ps, wk8, wkr8, x8t, xr8t, 0, HD, SL)
            norm_rope(k_ps, kT[:, js * SL:(js + 1) * SL], js, None)

            # V in natural [token, hd] layout: 4 token-subtile accumulation
            # groups sharing one PSUM bank (disjoint spans), N=128
            v_ps = ps_vo.tile([P, SL // P, HD], f32, tag="vo")
            for t in range(SL // P):
                terms = [(wv8, x8t), (wvr8, x8t), (wv8, xr8t)]
                for ti, (wt, xt) in enumerate(terms):
                    for pr in range(NPAIR):
                        nc.tensor.matmul(
                            v_ps[:, t, :],
                            xt[:, 2 * pr:2 * pr + 2, t * P:(t + 1) * P],
                            wt[:, 2 * pr:2 * pr + 2, :],
                            start=(ti, pr) == (0, 0),
                            stop=(ti, pr) == (2, NPAIR - 1),
                            perf_mode=DR)
            nc.scalar.copy(
                v_sb[:, js * (SL // P):(js + 1) * (SL // P), :], v_ps[:])

            for h in range(G):
                q_ps = ps_big.tile([P, SL], f32, tag="big", name=f"q_ps{js}_{h}")
                qkv_group(q_ps, wq8, wqr8, x8t, xr8t, h * HD, (h + 1) * HD, SL)
                norm_rope(q_ps, qT[h][:, js * SL:(js + 1) * SL], js,
                          qgain[:, h:h + 1])

            # ============ attention jq=js (+ proj js-1 filler) ============
            jq = js
            filler = ([(jq - 1, st, os_) for st in range(SL // P)
                       for os_ in range(D // SL)] if jq > 0 else [])
            nsteps = G * (4 * jq + 4)
            pace = max(1, nsteps // max(len(filler), 1))
            step = 0
            fi = 0
            ilast = 4 * jq + 3
            for h in range(G):
                o_ps = ps_o.tile([P, SL], f32, tag="o")
                pt_sum = ptsp.tile([P, SL], bf16, tag="ptsum")
                for i in range(ilast + 1):
                    delta = i - 4 * jq
                    # diagonal blocks: restrict to causally-relevant span,
                    # padded to >= 256 columns (f32r matmuls need N>=256
                    # for full rate); base shifts the causal select.
                    if 0 <= delta <= 3:
                        lo_c = min(P * delta, SL - 256)
                        base = lo_c - P * delta
                    else:
                        lo_c, base = 0, 0
                    sp = slice(lo_c, SL)
                    qsp = slice(jq * SL + lo_c, (jq + 1) * SL)
                    sc = ps_sc.tile([P, SL], f32, tag="sc")
                    nc.tensor.matmul(sc[:, sp], kT[:, i * P:(i + 1) * P],
                                     qT[h][:, qsp], start=True, stop=True)
                    pt = ptp.tile([P, SL], bf16, tag="pt")
                    nc.scalar.activation(pt[:, sp], sc[:, sp], AF.Exp)
                    if 0 <= delta <= 3:
                        nc.gpsimd.affine_select(
                            out=pt[:, sp], in_=pt[:, sp],
                            compare_op=OP.is_ge, fill=0.0,
                            base=base, pattern=[[1, SL - lo_c]],
                            channel_multiplier=-1)
                    if i == 0:
                        nc.vector.tensor_copy(pt_sum[:], pt[:])
                    else:
                        nc.vector.tensor_add(pt_sum[:, sp], pt_sum[:, sp],
                                             pt[:, sp])
                    nc.tensor.matmul(o_ps[:, sp], v_sb[:, i, :], pt[:, sp],
                                     start=(i == 0), stop=(i == ilast))
                    step += 1
                    if fi < len(filler) and step % pace == 0:
                        proj_chunk(*filler[fi])
                        fi += 1
                # rowsum via single matmul on the DVE-accumulated pt_sum
                rs = ps_nm.tile([P, SL], f32, tag="nm", name=f"rs{js}_{h}")
                nc.tensor.matmul(rs[:], ones_b[:], pt_sum[:],
                                 start=True, stop=True)
                rb = rbp.tile([P, SL], f32, tag="rb")
                nc.vector.reciprocal(rb[:], rs[:])
                nc.vector.tensor_mul(
                    oT[h][:, jq * SL:(jq + 1) * SL], o_ps[:], rb[:])
            while fi < len(filler):
                proj_chunk(*filler[fi])
                fi += 1

        # ---- final projection slice (jq=3) ----
        for st in range(SL // P):
            for os_ in range(D // SL):
                proj_chunk(NSL - 1, st, os_)

    nc.compile()
    return nc


def _rope_tables():
    """cos/sin tables in [HD, S] layout (half-tables stacked twice), plus J^T."""
    inv_freq = 1.0 / (ROPE_BASE ** (np.arange(0, HD, 2, dtype=np.float32) / HD))
    freqs = np.outer(np.arange(S, dtype=np.float32), inv_freq)  # [S, half]
    c = np.cos(freqs).T.astype(np.float32)  # [half, S]
    s = np.sin(freqs).T.astype(np.float32)
    cosf = np.concatenate([c, c], axis=0).copy()  # [HD, S]
    sinf = np.concatenate([s, s], axis=0).copy()
    half = HD // 2
    jT = np.zeros((HD, HD), np.float32)
    jT[np.arange(half) + half, np.arange(half)] = 1.0   # (Jq)[j] = q[j+64], j<64
    jT[np.arange(half), np.arange(half) + half] = -1.0  # (Jq)[j+64] = -q[j]
    return cosf, sinf, jT


def _q8pair(a, scale):
    """Host-side e4m3 two-level quantization of a*scale."""
    import ml_dtypes
    E4 = ml_dtypes.float8_e4m3
    xs = (np.asarray(a, np.float32) * scale).astype(np.float32)
    x8 = xs.astype(E4)
    xr8 = (xs - x8.astype(np.float32)).astype(E4)
    return np.ascontiguousarray(x8), np.ascontiguousarray(xr8)


def make_in_maps(x, Wq, Wk, Wv, Wproj, q_gain):
    """Host-side shard prep: per-core input dicts."""
    import ml_dtypes
    cosT, sinT, jT = _rope_tables()
    xT = np.transpose(np.asarray(x, np.float32), (0, 2, 1))
    x8 = [None] * B
    xr8 = [None] * B
    for b in range(B):
        x8[b], xr8[b] = _q8pair(xT[b], SX)
    Wq = np.asarray(Wq, np.float32)
    Wk = np.asarray(Wk, np.float32)
    Wv = np.asarray(Wv, np.float32)
    # v carries the 2^13 fp8 product scale; fold the inverse into Wproj
    WpT = (np.asarray(Wproj, np.float32).T / SXW).astype(ml_dtypes.bfloat16)
    q_gain = np.asarray(q_gain, np.float32)

    in_maps = []
    for c in range(NCORES):
        b, g = divmod(c, KVH)
        sl_q = slice(g * G * HD, (g + 1) * G * HD)
        sl_kv = slice(g * HD, (g + 1) * HD)
        wq8, wqr8 = _q8pair(Wq[sl_q, :].T, SW)
        wk8, wkr8 = _q8pair(Wk[sl_kv, :].T, SW)
        wv8, wvr8 = _q8pair(Wv[sl_kv, :].T, SW)
        in_maps.append({
            "x8": x8[b],
            "xr8": xr8[b],
            "wq8": wq8, "wqr8": wqr8,
            "wk8": wk8, "wkr8": wkr8,
            "wv8": wv8, "wvr8": wvr8,
            "wpT": np.ascontiguousarray(WpT[sl_q, :]),
            "cosT": cosT,
            "sinT": sinT,
            "jT": jT,
            "qgain": (q_gain[g * G:(g + 1) * G] / np.sqrt(HD))
            .reshape(1, G).astype(np.float32),
        })
    return in_maps


def kernel(x, Wq, Wk, Wv, Wproj, q_gain):
    from concourse.bass_utils import run_bass_kernel_spmd

    if "nc" not in _CACHE:
        _CACHE["nc"] = _build_program()
    nc = _CACHE["nc"]

    in_maps = make_in_maps(x, Wq, Wk, Wv, Wproj, q_gain)
    res = run_bass_kernel_spmd(nc, in_maps, core_ids=list(range(NCORES)))
    _CACHE["last_results"] = res

    y = np.zeros((B, S, D), dtype=np.float32)
    for c in range(NCORES):
        y[c // KVH] += res.results[c]["y"]
    return y
